# revision 1
# baseline (speedup 1.0000x reference)
"""Trainium2 Bass kernel for a custom LSTM layer (S=512, B=32, D=H=1024).

Strategy: hidden-dimension sharding across 8 NeuronCores (NOT the batch
data-parallel hint — per-step PE cost of the recurrence is set by streaming
V through the PE and is independent of batch, so batch sharding wins
nothing; splitting V's columns 8x does).

  - Core k owns hidden slice [128k, 128k+128) -> 4 gate-column strips of V/U.
  - Projection phase: xU^T (+bias) for all (t, b) precomputed into SBUF as
    bf16 at full PE utilization.
  - Recurrence phase (512 sequential steps): gates^T[4x128, 32] =
    sum_j V^T-tile @ h^T-chunk (V stationary, bf16, PSUM fp32), elementwise
    in [hidden, batch] orientation (full 128 partitions), then the 8 cores
    exchange their h^T slices with a per-step AllGather collective.
  - c kept fp32; h produced fp32 for outputs, bf16 for matmul/exchange.

Everything is laid out [hidden(partition), batch(free)] so no transposes
appear anywhere on the critical path; host-side numpy does all layout prep.
"""

from contextlib import ExitStack

import numpy as np

S, B, D, H, R = 512, 32, 1024, 1024, 8
GP = [0, 1, 3, 2]  # torch gate order (i,f,g,o) -> ours (i,f,o,g)
NC_CHUNK = 512  # projection n-chunk (columns of S*B per psum tile)

_CACHE: dict = {}
LAST_RESULT = None


def _build_nc():
    import concourse.bacc as bacc
    import concourse.mybir as mybir
    import concourse.tile as tile

    f32 = mybir.dt.float32
    bf16 = mybir.dt.bfloat16
    AF = mybir.ActivationFunctionType

    nc = bacc.Bacc(None, target_bir_lowering=False, num_devices=R)

    xt_d = nc.declare_dram_parameter("xt", [128, 8 * S * B], bf16, isOutput=False)
    ut_d = nc.declare_dram_parameter("ut", [128, 32 * 128], bf16, isOutput=False)
    vt_d = nc.declare_dram_parameter("vt", [128, 32 * 128], bf16, isOutput=False)
    bias_d = nc.declare_dram_parameter("bias", [128, 4], f32, isOutput=False)
    h0_d = nc.declare_dram_parameter("h0", [128, 8 * B], bf16, isOutput=False)
    c0_d = nc.declare_dram_parameter("c0", [128, B], f32, isOutput=False)
    allh_d = nc.declare_dram_parameter("allh", [S, 128, B], f32, isOutput=True)
    allc_d = nc.declare_dram_parameter("allc", [S, 128, B], f32, isOutput=True)

    with tile.TileContext(nc) as tc, ExitStack() as ctx:
        const = ctx.enter_context(tc.tile_pool(name="const", bufs=1))
        vt_sb = const.tile([128, 32 * 128], bf16, name="vt_sb")
        bias_sb = const.tile([128, 4], f32, name="bias_sb")
        xub = const.tile([128, 4 * S * B], bf16, name="xub")
        nc.sync.dma_start(vt_sb[:], vt_d[:, :])
        nc.sync.dma_start(bias_sb[:], bias_d[:, :])

        # ---------------- projection: xub[p, g, n] = (x @ U^T + b)^T --------
        nch_total = S * B // NC_CHUNK
        with (
            tc.tile_pool(name="putl", bufs=1) as putl,
            tc.tile_pool(name="proj", bufs=3) as proj,
            tc.tile_pool(name="ppsum", bufs=4, space="PSUM") as ppsum,
        ):
            ut_sb = putl.tile([128, 32 * 128], bf16, name="ut_sb")
            nc.sync.dma_start(ut_sb[:], ut_d[:, :])
            xt_view = xt_d[:, :].rearrange("p (j n) -> p j n", j=8)
            for nch in range(nch_total):
                n0 = nch * NC_CHUNK
                xt_t = proj.tile([128, 8, NC_CHUNK], bf16, name="xt_t", tag="xt_t")
                nc.sync.dma_start(xt_t[:], xt_view[:, :, n0 : n0 + NC_CHUNK])
                for g in range(4):
                    ps = ppsum.tile([128, NC_CHUNK], f32, name="ps", tag="ps")
                    for j in range(8):
                        w0 = (g * 8 + j) * 128
                        nc.tensor.matmul(
                            ps[:, :],
                            ut_sb[:, w0 : w0 + 128],
                            xt_t[:, j, :],
                            start=(j == 0),
                            stop=(j == 7),
                        )
                    nc.scalar.activation(
                        xub[:, g * S * B + n0 : g * S * B + n0 + NC_CHUNK],
                        ps[:, :],
                        AF.Identity,
                        bias=bias_sb[:, g : g + 1],
                    )

        # ---------------- recurrence ----------------------------------------
        with (
            tc.tile_pool(name="rec", bufs=3) as rec,
            tc.tile_pool(name="rpsum", bufs=2, space="PSUM") as rpsum,
            tc.tile_pool(name="dram", bufs=2, space="DRAM") as dram,
        ):
            hT = rec.tile([128, 8, B], bf16, name="hT_init", tag="hT")
            nc.sync.dma_start(hT[:], h0_d[:, :].rearrange("p (j b) -> p j b", j=8))
            cT = rec.tile([128, B], f32, name="cT_init", tag="cT")
            nc.sync.dma_start(cT[:], c0_d[:, :])

            xub_v = xub[:, :].rearrange("p (g n) -> p g n", g=4)
            for t in range(S):
                ps = rpsum.tile([128, 4 * 512], f32, name="ps2", tag="ps2")
                for g in range(4):
                    for j in range(8):
                        w0 = (g * 8 + j) * 128
                        nc.tensor.matmul(
                            ps[:, g * 512 : g * 512 + B],
                            vt_sb[:, w0 : w0 + 128],
                            hT[:, j, :],
                            start=(j == 0),
                            stop=(j == 7),
                        )
                pre = rec.tile([128, 4, B], f32, name="pre", tag="pre")
                nc.vector.tensor_add(
                    pre[:],
                    ps[:, :].rearrange("p (g n) -> p g n", g=4)[:, :, 0:B],
                    xub_v[:, :, t * B : (t + 1) * B],
                )
                pre_f = pre[:].rearrange("p g n -> p (g n)")
                acts = rec.tile([128, 4 * B], f32, name="acts", tag="acts")
                nc.scalar.activation(
                    acts[:, 0 : 3 * B], pre_f[:, 0 : 3 * B], AF.Sigmoid
                )
                nc.scalar.activation(
                    acts[:, 3 * B : 4 * B], pre_f[:, 3 * B : 4 * B], AF.Tanh
                )
                ig = rec.tile([128, B], f32, name="ig", tag="ig")
                nc.vector.tensor_mul(ig[:], acts[:, 0:B], acts[:, 3 * B : 4 * B])
                c_new = rec.tile([128, B], f32, name="c_new", tag="cT")
                nc.vector.tensor_mul(c_new[:], acts[:, B : 2 * B], cT[:])
                nc.vector.tensor_add(c_new[:], c_new[:], ig[:])
                nc.gpsimd.dma_start(allc_d[t, :, :], c_new[:])
                tanh_c = rec.tile([128, B], f32, name="tanh_c", tag="tanh_c")
                nc.scalar.activation(tanh_c[:], c_new[:], AF.Tanh)
                h_new = rec.tile([128, B], f32, name="h_new", tag="h_new")
                nc.vector.tensor_mul(h_new[:], acts[:, 2 * B : 3 * B], tanh_c[:])
                nc.gpsimd.dma_start(allh_d[t, :, :], h_new[:])
                h_bf = rec.tile([128, B], bf16, name="h_bf", tag="h_bf")
                nc.vector.tensor_copy(h_bf[:], h_new[:])

                # exchange: AllGather of this core's h^T slice
                bi = dram.tile([128, B], bf16, name="bi", tag="bi")
                bo = dram.tile([R, 128, B], bf16, name="bo", tag="bo")
                nc.sync.dma_start(bi[:], h_bf[:])
                nc.gpsimd.collective_compute(
                    "AllGather",
                    mybir.AluOpType.bypass,
                    replica_groups=[list(range(R))],
                    ins=[bi[:, :].opt()],
                    outs=[bo[:, :, :].opt()],
                )
                hT = rec.tile([128, 8, B], bf16, name="hT", tag="hT")
                nc.sync.dma_start(hT[:], bo[:, :, :].rearrange("j p b -> p j b"))
                cT = c_new
    nc.compile()
    return nc


def _get_nc():
    if "nc" not in _CACHE:
        _CACHE["nc"] = _build_nc()
    return _CACHE["nc"]


def kernel(layer_input, h_t, c_t, U, V, bih, bhh):
    import ml_dtypes

    from concourse.bass_utils import run_bass_kernel_spmd

    global LAST_RESULT
    bf16 = ml_dtypes.bfloat16
    X = np.asarray(layer_input, np.float32)
    h_t = np.asarray(h_t, np.float32)
    c_t = np.asarray(c_t, np.float32)
    U = np.asarray(U, np.float32)
    V = np.asarray(V, np.float32)
    bias = (np.asarray(bih, np.float32) + np.asarray(bhh, np.float32)).reshape(
        4, H
    )[GP]

    U4 = U.reshape(4, H, D)[GP]  # [g, row, d]
    V4 = V.reshape(4, H, H)[GP]

    # xt[p, j, n] = X[t, b, 128j+p], n = t*B + b  (shared by all cores)
    Xf = X.reshape(S * B, D)
    xt = (
        np.ascontiguousarray(Xf.T.reshape(8, 128, S * B).transpose(1, 0, 2))
        .reshape(128, 8 * S * B)
        .astype(bf16)
    )
    # h0[p, j, b] = h_t[b, 128j+p]  (shared)
    h0 = (
        np.ascontiguousarray(h_t.T.reshape(8, 128, B).transpose(1, 0, 2))
        .reshape(128, 8 * B)
        .astype(bf16)
    )

    in_maps = []
    for k in range(R):
        sl = slice(128 * k, 128 * k + 128)
        # ut[p, g, j, c] = U4[g, 128k+c, 128j+p]
        ut = (
            U4[:, sl, :]
            .reshape(4, 128, 8, 128)
            .transpose(3, 0, 2, 1)
            .reshape(128, 32 * 128)
            .astype(bf16)
        )
        vt = (
            V4[:, sl, :]
            .reshape(4, 128, 8, 128)
            .transpose(3, 0, 2, 1)
            .reshape(128, 32 * 128)
            .astype(bf16)
        )
        bias_k = np.ascontiguousarray(bias[:, sl].T)  # [128, 4]
        c0 = np.ascontiguousarray(c_t.T[sl, :])  # [128, B]
        in_maps.append(
            {
                "xt": xt,
                "ut": np.ascontiguousarray(ut),
                "vt": np.ascontiguousarray(vt),
                "bias": bias_k,
                "h0": h0,
                "c0": c0,
            }
        )

    nc = _get_nc()
    res = run_bass_kernel_spmd(nc, in_maps, list(range(R)))
    LAST_RESULT = res

    all_h = np.empty((S, B, H), np.float32)
    all_c = np.empty((S, B, H), np.float32)
    for k in range(R):
        sl = slice(128 * k, 128 * k + 128)
        all_h[:, :, sl] = res.results[k]["allh"].transpose(0, 2, 1)
        all_c[:, :, sl] = res.results[k]["allc"].transpose(0, 2, 1)
    h_f = all_h[-1].copy()
    c_f = all_c[-1].copy()
    return (all_h, h_f, c_f, all_h, all_c)
